# revision 33
# baseline (speedup 1.0000x reference)
"""Local (sliding-window, causal) attention on 8 Trainium2 NeuronCores.

Problem: B=8, L=4096, H=8, E=64, window NEIGH=128, SPLITS=32 query blocks of
L1=128.  Query q attends keys [q-127, q].

Sharding: batch b -> core b (8 cores, no communication).

Per-core algorithm (streaming over the 32 sequence tiles):
  - Host pre-packs (numpy): Q,K cast to bf16 and transposed to [e, l] layout
    (partition = e for head h at free offset h*128, free = l), V cast to bf16
    with a ones-column appended per head (denominator trick).
  - ST scores computed transposed [m, l] so that softmax needs NO partition
    reduction and P is consumed by the AV matmul WITHOUT a transpose:
       ST = K_tile @ Q_block^T  (PE, bf16, contraction e=64)
       P = exp(ST * 0.125) (ACT, -> bf16)  [no max-subtraction needed: |S|<~8]
       P *= band mask (0/1 bf16; heads 0-3 on DVE, 4-7 on POOL)
       out_aug[l, 0:65] = sum over the two m-tiles of P^T @ [V | ones]  (PE)
       out[l, 0:64] = out_aug[:, 0:64] / out_aug[:, 64]  (DVE, direct from
       PSUM, emitted as fp16 to halve the output DMA)
  - Block j needs key tiles j-1, j; per step t we compute the single matmul
    pair (stationary KT_{t-1}) that yields tile-b scores of block t-1 and
    tile-a scores of block t; block t-1 then completes and is stored.
"""

import numpy as np
import ml_dtypes

B, L, H, E = 8, 4096, 8, 64
NEIGH = 128
P = 128                 # partitions / rows per tile
T = L // P              # 32 sequence tiles
N_CORES = 8
SCALE = 1.0 / np.sqrt(E)
BF = ml_dtypes.bfloat16

_CACHE = {}


def build_bass(nsteps=T):
    """Build + compile the single-core Bass program (SPMD across 8 cores)."""
    from contextlib import ExitStack
    import concourse.bass as bass  # noqa: F401
    import concourse.mybir as mybir
    import concourse.tile as tile
    from concourse import bacc

    f32, bf16 = mybir.dt.float32, mybir.dt.bfloat16
    f16 = mybir.dt.float16
    Exp = mybir.ActivationFunctionType.Exp

    nc = bacc.Bacc(
        "TRN2", target_bir_lowering=False, debug=False, enable_asserts=False
    )
    qkt_d = nc.dram_tensor(
        "qkt", [nsteps, E, 2 * H * P], bf16, kind="ExternalInput"
    ).ap()
    va_d = nc.dram_tensor(
        "va", [nsteps, P, H * (E + 1)], bf16, kind="ExternalInput"
    ).ap()
    mk_d = nc.dram_tensor("mk", [P, 2 * P], bf16, kind="ExternalInput").ap()
    out_d = nc.dram_tensor("out", [nsteps, P, H * E], f16, kind="ExternalOutput").ap()

    with tile.TileContext(nc) as tc:
        with ExitStack() as ctx:
            nc = tc.nc

            const = ctx.enter_context(tc.tile_pool(name="const", bufs=1))
            # multiplicative band mask, replicated per head: [128, H*256] bf16
            # per head: [0:128] tile-b (valid l>=m), [128:256] tile-a (valid l<m)
            mask = const.tile([P, H * 2 * P], bf16, tag="mask")
            mv = mask[:].rearrange("p (r w) -> p r w", r=H)
            # one DMA, source re-read H times via stride-0 broadcast; issued
            # from ACT (busy with its table load anyway) to keep SP and POOL
            # free for the first qkt/va loads
            nc.scalar.dma_start(
                mv[:],
                mk_d[:]
                .rearrange("p (h w) -> p h w", h=1)
                .broadcast_to([P, H, 2 * P]),
            )

            qk = ctx.enter_context(tc.tile_pool(name="qk", bufs=4))
            vp = ctx.enter_context(tc.tile_pool(name="vp", bufs=4))
            pp = ctx.enter_context(tc.tile_pool(name="pp", bufs=3))
            op = ctx.enter_context(tc.tile_pool(name="op", bufs=4))
            rp = ctx.enter_context(tc.tile_pool(name="rp", bufs=4))
            st_ps = ctx.enter_context(tc.tile_pool(name="st", bufs=2, space="PSUM"))
            av_ps = ctx.enter_context(tc.tile_pool(name="av", bufs=2, space="PSUM"))

            qkt_prev = None
            p_prev = None
            va_hist = [None, None]  # [V tile t-1, V tile t-2]

            for t in range(nsteps + 1):
                qkt = va = None
                if t < nsteps:
                    qkt = qk.tile([E, 2 * H * P], bf16, tag="qkt")
                    # t==1 from POOL so the first two loads overlap (POOL has
                    # no compute until step 1's mask-multiply)
                    eng = nc.gpsimd if t == 1 else nc.sync
                    eng.dma_start(qkt[:], qkt_d[t])
                    va = vp.tile([P, H * (E + 1)], bf16, tag="va")
                    nc.gpsimd.dma_start(va[:], va_d[t])

                if t >= 1:
                    # scores for (block t-1 | tile-b) and (block t | tile-a)
                    pt = pp.tile([P, H * 2 * P], bf16, tag="pt")
                    for g in range(2):  # two groups of 4 heads
                        st = st_ps.tile([P, 4 * 2 * P], f32, tag="st")
                        for i in range(4):
                            h = g * 4 + i
                            c0, c1 = h * P, (h + 1) * P
                            lh = qkt_prev[:, H * P + c0 : H * P + c1]
                            nc.tensor.matmul(
                                st[:, i * 2 * P : i * 2 * P + P],
                                lh, qkt_prev[:, c0:c1],
                                start=True, stop=True,
                            )
                            if t < nsteps:
                                nc.tensor.matmul(
                                    st[:, i * 2 * P + P : (i + 1) * 2 * P],
                                    lh, qkt[:, c0:c1],
                                    start=True, stop=True,
                                )
                        if t < nsteps:
                            nc.scalar.activation(
                                pt[:, g * 4 * 2 * P : (g + 1) * 4 * 2 * P],
                                st[:], Exp, scale=float(SCALE),
                            )
                        else:
                            # last step: only tile-b (left) halves were
                            # written; one strided activation per group
                            sv = st[:].rearrange("p (r w) -> p r w", r=4)
                            gv = (
                                pt[:, g * 4 * 2 * P : (g + 1) * 4 * 2 * P]
                                .rearrange("p (r w) -> p r w", r=4)
                            )
                            nc.scalar.activation(
                                gv[:, :, 0:P], sv[:, :, 0:P],
                                Exp, scale=float(SCALE),
                            )
                    # band mask: heads 0-5 on DVE, heads 6-7 on POOL
                    cut = 6 * 2 * P
                    if t < nsteps:
                        nc.vector.tensor_mul(
                            pt[:, 0:cut], pt[:, 0:cut], mask[:, 0:cut]
                        )
                        nc.gpsimd.tensor_mul(
                            pt[:, cut:], pt[:, cut:], mask[:, cut:]
                        )
                    else:
                        pv = pt[:].rearrange("p (r w) -> p r w", r=H)
                        nc.vector.tensor_mul(
                            pv[:, 0:6, 0:P], pv[:, 0:6, 0:P], mv[:, 0:6, 0:P]
                        )
                        nc.gpsimd.tensor_mul(
                            pv[:, 6:H, 0:P], pv[:, 6:H, 0:P], mv[:, 6:H, 0:P]
                        )

                    # AV for block j = t-1  (out_aug per head: 64 V cols + denom)
                    av = av_ps.tile([P, H * P], f32, tag="av")  # head h at h*128
                    for h in range(H):
                        dst = av[:, h * P : h * P + (E + 1)]
                        vs1 = va_hist[0][:, h * (E + 1) : (h + 1) * (E + 1)]
                        if t >= 2:
                            vs2 = va_hist[1][:, h * (E + 1) : (h + 1) * (E + 1)]
                            nc.tensor.matmul(
                                dst, p_prev[:, h * 2 * P + P : (h + 1) * 2 * P],
                                vs2, start=True, stop=False,
                            )
                            nc.tensor.matmul(
                                dst, pt[:, h * 2 * P : h * 2 * P + P],
                                vs1, start=False, stop=True,
                            )
                        else:
                            nc.tensor.matmul(
                                dst, pt[:, h * 2 * P : h * 2 * P + P],
                                vs1, start=True, stop=True,
                            )

                    # out = av[:, 0:64] / av[:, 64], straight from PSUM -> fp16
                    avv = av[:].rearrange("p (h w) -> p h w", h=H)
                    rr = rp.tile([P, H], f32, tag="rr")
                    rrv = rr[:].rearrange("p (h w) -> p h w", w=1)
                    nc.vector.reciprocal(rrv, avv[:, :, E : E + 1])
                    ob = op.tile([P, H * E], f16, tag="ob")
                    obv = ob[:].rearrange("p (h w) -> p h w", h=H)
                    nc.vector.tensor_mul(
                        obv, avv[:, :, 0:E], rrv.broadcast_to([P, H, E])
                    )
                    nc.gpsimd.dma_start(out_d[t - 1], ob[:])
                    p_prev = pt

                if t < nsteps:
                    va_hist = [va, va_hist[0]]
                    qkt_prev = qkt

    nc.compile()
    return nc


def make_mask():
    """[P, 2P] bf16: [0:128] tile-b valid l>=m; [128:256] tile-a valid l<m."""
    m = np.arange(P)[:, None]
    l = np.arange(P)[None, :]
    mb = (l >= m).astype(np.float32)
    ma = (l < m).astype(np.float32)
    return np.concatenate([mb, ma], axis=1).astype(BF)


def pack_inputs(q, k, v):
    """Per-core host repack: q,k,v [L, H, E] f32 -> dict of device inputs."""
    nst = q.shape[0] // P

    def t_pack(x):
        xb = np.ascontiguousarray(
            x.reshape(nst, P, H, E).transpose(0, 3, 2, 1)
        )  # [t, e, h, l]
        return xb.reshape(nst, E, H * P).astype(BF)

    qkt = np.concatenate([t_pack(q), t_pack(k)], axis=-1)
    vb = v.reshape(nst, P, H, E).astype(BF)
    va = np.concatenate(
        [vb, np.ones((nst, P, H, 1), BF)], axis=-1
    ).reshape(nst, P, H * (E + 1))
    return {"qkt": qkt, "va": va, "mk": make_mask()}


def unpack_output(out, L_):
    """Device out [nsteps, P, H*E] fp16 -> [L, H, E] f32."""
    return np.asarray(out, np.float32).reshape(L_, H, E)


def _ensure_fast_setup(nc, n_cores):
    """Build + cache the sharded executable, on-device zeros maker, and
    name/mesh metadata for the fast PJRT path."""
    import jax
    import jax.numpy as jnp
    from jax.experimental.shard_map import shard_map
    from jax.sharding import Mesh, NamedSharding, PartitionSpec
    from concourse import bass2jax, mybir

    bass2jax.install_neuronx_cc_hook()

    key = id(nc)
    if _CACHE.get("fast_key") != key:
        partition_name = (
            nc.partition_id_tensor.name if nc.partition_id_tensor else None
        )
        in_names, out_names, out_avals, zero_shapes = [], [], [], []
        for alloc in nc.m.functions[0].allocations:
            if not isinstance(alloc, mybir.MemoryLocationSet):
                continue
            name = alloc.memorylocations[0].name
            if alloc.kind == "ExternalInput":
                if name != partition_name:
                    in_names.append(name)
            elif alloc.kind == "ExternalOutput":
                shape = tuple(alloc.tensor_shape)
                dtype = mybir.dt.np(alloc.dtype)
                out_names.append(name)
                out_avals.append(jax.core.ShapedArray(shape, dtype))
                zero_shapes.append((shape, dtype))
        n_params = len(in_names)
        n_outs = len(out_avals)
        in_names.extend(out_names)
        if partition_name is not None:
            in_names.append(partition_name)
        donate = tuple(range(n_params, n_params + n_outs))

        def _body(*args):
            operands = list(args)
            if partition_name is not None:
                operands.append(bass2jax.partition_id_tensor())
            outs = bass2jax._bass_exec_p.bind(
                *operands,
                out_avals=tuple(out_avals),
                in_names=tuple(in_names),
                out_names=tuple(out_names),
                lowering_input_output_aliases=(),
                sim_require_finite=True,
                sim_require_nnan=True,
                nc=nc,
            )
            return tuple(outs)

        devices = jax.devices()[:n_cores]
        mesh = Mesh(np.asarray(devices), ("core",))
        sharded = jax.jit(
            shard_map(
                _body,
                mesh=mesh,
                in_specs=(PartitionSpec("core"),) * (n_params + n_outs),
                out_specs=(PartitionSpec("core"),) * n_outs,
                check_rep=False,
            ),
            donate_argnums=donate,
            keep_unused=True,
        )
        zsh = (NamedSharding(mesh, PartitionSpec("core")),) * n_outs
        mk_zeros = jax.jit(
            lambda: tuple(
                jnp.zeros((n_cores * s[0], *s[1:]), d) for s, d in zero_shapes
            ),
            out_shardings=zsh,
        )
        _CACHE.update(
            fast_key=key, fast_sharded=sharded, fast_mk_zeros=mk_zeros,
            fast_names=(in_names, out_names, out_avals, n_params),
            fast_devices=devices, fast_sharding=zsh[0] if zsh else None,
        )


def _fast_run_bass_via_pjrt(nc, in_maps, n_cores):
    """Drop-in for bass2jax.run_bass_via_pjrt (multi-core, no-debug path):
    donated zero output buffers are allocated ON DEVICE, and per-core input
    shards that are already jax arrays (pre-staged asynchronously by
    kernel()) are assembled without a host-side concat."""
    import jax

    if n_cores == 1 or nc.dbg_addr is not None:
        return _CACHE["orig_run"](nc, in_maps, n_cores)

    _ensure_fast_setup(nc, n_cores)
    in_names, out_names, out_avals, n_params = _CACHE["fast_names"]
    sharded = _CACHE["fast_sharded"]
    sharding = _CACHE["fast_sharding"]

    if isinstance(in_maps[0][in_names[0]], jax.Array):
        global_in = []
        for name in in_names[:n_params]:
            shards = [in_maps[c][name] for c in range(n_cores)]
            s0 = shards[0].shape
            global_in.append(
                jax.make_array_from_single_device_arrays(
                    (n_cores * s0[0], *s0[1:]), sharding, shards
                )
            )
    else:
        global_in = [
            np.concatenate(
                [np.asarray(in_maps[c][name]) for c in range(n_cores)], axis=0
            )
            for name in in_names[:n_params]
        ]
    concat_zeros = _CACHE["fast_mk_zeros"]()
    out_arrs = sharded(*global_in, *concat_zeros)
    try:
        # fetch the 8 per-device shards concurrently (~1.3x tunnel speedup)
        from concurrent.futures import ThreadPoolExecutor

        results = [dict() for _ in range(n_cores)]
        for i, name in enumerate(out_names):
            shards = sorted(
                out_arrs[i].addressable_shards,
                key=lambda s: s.index[0].start or 0,
            )
            assert len(shards) == n_cores
            with ThreadPoolExecutor(n_cores) as ex:
                datas = list(ex.map(lambda s: np.asarray(s.data), shards))
            for c in range(n_cores):
                results[c][name] = datas[c]
        return results
    except Exception:
        return [
            {
                name: np.asarray(out_arrs[i]).reshape(
                    n_cores, *out_avals[i].shape
                )[c]
                for i, name in enumerate(out_names)
            }
            for c in range(n_cores)
        ]


def _install_fast_pjrt():
    from concourse import bass2jax

    if "orig_run" not in _CACHE:
        _CACHE["orig_run"] = bass2jax.run_bass_via_pjrt
        bass2jax.run_bass_via_pjrt = _fast_run_bass_via_pjrt


def kernel(queries, keys, values):
    from concourse import bass_utils

    if "nc" not in _CACHE:
        _CACHE["nc"] = build_bass(T)
    nc = _CACHE["nc"]

    in_maps = None
    try:
        # pre-stage: pack each core then start its h2d transfer on a thread
        # pool, so host packing and the (slow, ~1.3x-parallelizable) tunnel
        # transfers overlap
        from concurrent.futures import ThreadPoolExecutor

        import jax

        _install_fast_pjrt()
        _ensure_fast_setup(nc, N_CORES)
        devs = _CACHE["fast_devices"]

        def _put(b, k2, v2):
            a = jax.device_put(v2, devs[b])
            a.block_until_ready()
            return b, k2, a

        futs = []
        with ThreadPoolExecutor(6) as ex:
            for b in range(N_CORES):
                m = pack_inputs(
                    np.asarray(queries[b]), np.asarray(keys[b]),
                    np.asarray(values[b]),
                )
                for k2, v2 in m.items():
                    futs.append(ex.submit(_put, b, k2, v2))
            in_maps = [dict() for _ in range(N_CORES)]
            for f in futs:
                b, k2, a = f.result()
                in_maps[b][k2] = a
    except Exception:
        in_maps = None

    if in_maps is None:
        in_maps = [
            pack_inputs(
                np.asarray(queries[b]), np.asarray(keys[b]),
                np.asarray(values[b]),
            )
            for b in range(N_CORES)
        ]

    try:
        _install_fast_pjrt()
        res = bass_utils.run_bass_kernel_spmd(
            nc, in_maps, core_ids=list(range(N_CORES))
        )
    except Exception:
        from concourse import bass2jax

        if "orig_run" in _CACHE:  # unpatch and retry on the stock path
            bass2jax.run_bass_via_pjrt = _CACHE["orig_run"]
        res = bass_utils.run_bass_kernel_spmd(
            nc, in_maps, core_ids=list(range(N_CORES))
        )
    out = np.stack(
        [unpack_output(res.results[b]["out"], L) for b in range(N_CORES)]
    )
    _CACHE["last_result"] = res
    return out.reshape(B, L, H, E)


# revision 34
# speedup vs baseline: 1.0423x; 1.0423x over previous
"""Local (sliding-window, causal) attention on 8 Trainium2 NeuronCores.

Problem: B=8, L=4096, H=8, E=64, window NEIGH=128, SPLITS=32 query blocks of
L1=128.  Query q attends keys [q-127, q].

Sharding: batch b -> core b (8 cores, no communication).

Per-core algorithm (streaming over the 32 sequence tiles):
  - Host pre-packs (numpy): Q,K cast to bf16 and transposed to [e, l] layout
    (partition = e for head h at free offset h*128, free = l), V cast to bf16
    with a ones-column appended per head (denominator trick).
  - ST scores computed transposed [m, l] so that softmax needs NO partition
    reduction and P is consumed by the AV matmul WITHOUT a transpose:
       ST = K_tile @ Q_block^T  (PE, bf16, contraction e=64)
       P = exp(ST * 0.125) (ACT, -> bf16)  [no max-subtraction needed: |S|<~8]
       P *= band mask (0/1 bf16; heads 0-3 on DVE, 4-7 on POOL)
       out_aug[l, 0:65] = sum over the two m-tiles of P^T @ [V | ones]  (PE)
       out[l, 0:64] = out_aug[:, 0:64] / out_aug[:, 64]  (DVE, direct from
       PSUM, emitted as fp16 to halve the output DMA)
  - Block j needs key tiles j-1, j; per step t we compute the single matmul
    pair (stationary KT_{t-1}) that yields tile-b scores of block t-1 and
    tile-a scores of block t; block t-1 then completes and is stored.
"""

import numpy as np
import ml_dtypes

B, L, H, E = 8, 4096, 8, 64
NEIGH = 128
P = 128                 # partitions / rows per tile
T = L // P              # 32 sequence tiles
N_CORES = 8
SCALE = 1.0 / np.sqrt(E)
BF = ml_dtypes.bfloat16

_CACHE = {}


def build_bass(nsteps=T):
    """Build + compile the single-core Bass program (SPMD across 8 cores)."""
    from contextlib import ExitStack
    import concourse.bass as bass  # noqa: F401
    import concourse.mybir as mybir
    import concourse.tile as tile
    from concourse import bacc

    f32, bf16 = mybir.dt.float32, mybir.dt.bfloat16
    f16 = mybir.dt.float16
    Exp = mybir.ActivationFunctionType.Exp

    nc = bacc.Bacc(
        "TRN2", target_bir_lowering=False, debug=False, enable_asserts=False
    )
    qkt_d = nc.dram_tensor(
        "qkt", [nsteps, E, 2 * H * P], bf16, kind="ExternalInput"
    ).ap()
    va_d = nc.dram_tensor(
        "va", [nsteps, P, H * (E + 1)], bf16, kind="ExternalInput"
    ).ap()
    mk_d = nc.dram_tensor("mk", [P, 2 * P], bf16, kind="ExternalInput").ap()
    out_d = nc.dram_tensor("out", [nsteps, P, H * E], f16, kind="ExternalOutput").ap()

    with tile.TileContext(nc) as tc:
        with ExitStack() as ctx:
            nc = tc.nc

            const = ctx.enter_context(tc.tile_pool(name="const", bufs=1))
            # multiplicative band mask, replicated per head: [128, H*256] bf16
            # per head: [0:128] tile-b (valid l>=m), [128:256] tile-a (valid l<m)
            mask = const.tile([P, H * 2 * P], bf16, tag="mask")
            mv = mask[:].rearrange("p (r w) -> p r w", r=H)
            # one DMA, source re-read H times via stride-0 broadcast; issued
            # from ACT (busy with its table load anyway) to keep SP and POOL
            # free for the first qkt/va loads
            nc.scalar.dma_start(
                mv[:],
                mk_d[:]
                .rearrange("p (h w) -> p h w", h=1)
                .broadcast_to([P, H, 2 * P]),
            )

            qk = ctx.enter_context(tc.tile_pool(name="qk", bufs=4))
            vp = ctx.enter_context(tc.tile_pool(name="vp", bufs=4))
            pp = ctx.enter_context(tc.tile_pool(name="pp", bufs=3))
            op = ctx.enter_context(tc.tile_pool(name="op", bufs=4))
            rp = ctx.enter_context(tc.tile_pool(name="rp", bufs=4))
            st_ps = ctx.enter_context(tc.tile_pool(name="st", bufs=2, space="PSUM"))
            av_ps = ctx.enter_context(tc.tile_pool(name="av", bufs=2, space="PSUM"))

            qkt_prev = None
            p_prev = None
            va_hist = [None, None]  # [V tile t-1, V tile t-2]

            for t in range(nsteps + 1):
                qkt = va = None
                if t < nsteps:
                    qkt = qk.tile([E, 2 * H * P], bf16, tag="qkt")
                    # t==1 from POOL so the first two loads overlap (POOL has
                    # no compute until step 1's mask-multiply)
                    eng = nc.gpsimd if t == 1 else nc.sync
                    eng.dma_start(qkt[:], qkt_d[t])
                    va = vp.tile([P, H * (E + 1)], bf16, tag="va")
                    nc.gpsimd.dma_start(va[:], va_d[t])

                if t >= 1:
                    # scores for (block t-1 | tile-b) and (block t | tile-a)
                    pt = pp.tile([P, H * 2 * P], bf16, tag="pt")
                    for g in range(2):  # two groups of 4 heads
                        st = st_ps.tile([P, 4 * 2 * P], f32, tag="st")
                        for i in range(4):
                            h = g * 4 + i
                            c0, c1 = h * P, (h + 1) * P
                            lh = qkt_prev[:, H * P + c0 : H * P + c1]
                            nc.tensor.matmul(
                                st[:, i * 2 * P : i * 2 * P + P],
                                lh, qkt_prev[:, c0:c1],
                                start=True, stop=True,
                            )
                            if t < nsteps:
                                nc.tensor.matmul(
                                    st[:, i * 2 * P + P : (i + 1) * 2 * P],
                                    lh, qkt[:, c0:c1],
                                    start=True, stop=True,
                                )
                        if t < nsteps:
                            nc.scalar.activation(
                                pt[:, g * 4 * 2 * P : (g + 1) * 4 * 2 * P],
                                st[:], Exp, scale=float(SCALE),
                            )
                        else:
                            # last step: only tile-b (left) halves were
                            # written; one strided activation per group
                            sv = st[:].rearrange("p (r w) -> p r w", r=4)
                            gv = (
                                pt[:, g * 4 * 2 * P : (g + 1) * 4 * 2 * P]
                                .rearrange("p (r w) -> p r w", r=4)
                            )
                            nc.scalar.activation(
                                gv[:, :, 0:P], sv[:, :, 0:P],
                                Exp, scale=float(SCALE),
                            )
                    # band mask: heads 0-5 on DVE, heads 6-7 on POOL
                    cut = 6 * 2 * P
                    if t < nsteps:
                        nc.vector.tensor_mul(
                            pt[:, 0:cut], pt[:, 0:cut], mask[:, 0:cut]
                        )
                        nc.gpsimd.tensor_mul(
                            pt[:, cut:], pt[:, cut:], mask[:, cut:]
                        )
                    else:
                        pv = pt[:].rearrange("p (r w) -> p r w", r=H)
                        nc.vector.tensor_mul(
                            pv[:, 0:6, 0:P], pv[:, 0:6, 0:P], mv[:, 0:6, 0:P]
                        )
                        nc.gpsimd.tensor_mul(
                            pv[:, 6:H, 0:P], pv[:, 6:H, 0:P], mv[:, 6:H, 0:P]
                        )

                    # AV for block j = t-1  (out_aug per head: 64 V cols + denom)
                    av = av_ps.tile([P, H * P], f32, tag="av")  # head h at h*128
                    for h in range(H):
                        dst = av[:, h * P : h * P + (E + 1)]
                        vs1 = va_hist[0][:, h * (E + 1) : (h + 1) * (E + 1)]
                        if t >= 2:
                            vs2 = va_hist[1][:, h * (E + 1) : (h + 1) * (E + 1)]
                            nc.tensor.matmul(
                                dst, p_prev[:, h * 2 * P + P : (h + 1) * 2 * P],
                                vs2, start=True, stop=False,
                            )
                            nc.tensor.matmul(
                                dst, pt[:, h * 2 * P : h * 2 * P + P],
                                vs1, start=False, stop=True,
                            )
                        else:
                            nc.tensor.matmul(
                                dst, pt[:, h * 2 * P : h * 2 * P + P],
                                vs1, start=True, stop=True,
                            )

                    # out = av[:, 0:64] / av[:, 64], straight from PSUM -> fp16
                    avv = av[:].rearrange("p (h w) -> p h w", h=H)
                    rr = rp.tile([P, H], f32, tag="rr")
                    rrv = rr[:].rearrange("p (h w) -> p h w", w=1)
                    nc.vector.reciprocal(rrv, avv[:, :, E : E + 1])
                    ob = op.tile([P, H * E], f16, tag="ob")
                    obv = ob[:].rearrange("p (h w) -> p h w", h=H)
                    nc.vector.tensor_mul(
                        obv, avv[:, :, 0:E], rrv.broadcast_to([P, H, E])
                    )
                    nc.gpsimd.dma_start(out_d[t - 1], ob[:])
                    p_prev = pt

                if t < nsteps:
                    va_hist = [va, va_hist[0]]
                    qkt_prev = qkt

    nc.compile()
    return nc


def make_mask():
    """[P, 2P] bf16: [0:128] tile-b valid l>=m; [128:256] tile-a valid l<m."""
    m = np.arange(P)[:, None]
    l = np.arange(P)[None, :]
    mb = (l >= m).astype(np.float32)
    ma = (l < m).astype(np.float32)
    return np.concatenate([mb, ma], axis=1).astype(BF)


def pack_inputs(q, k, v):
    """Per-core host repack: q,k,v [L, H, E] f32 -> dict of device inputs."""
    nst = q.shape[0] // P

    def t_pack(x):
        xb = np.ascontiguousarray(
            x.reshape(nst, P, H, E).transpose(0, 3, 2, 1)
        )  # [t, e, h, l]
        return xb.reshape(nst, E, H * P).astype(BF)

    qkt = np.concatenate([t_pack(q), t_pack(k)], axis=-1)
    vb = v.reshape(nst, P, H, E).astype(BF)
    va = np.concatenate(
        [vb, np.ones((nst, P, H, 1), BF)], axis=-1
    ).reshape(nst, P, H * (E + 1))
    return {"qkt": qkt, "va": va, "mk": make_mask()}


def unpack_output(out, L_):
    """Device out [nsteps, P, H*E] fp16 -> [L, H, E] f32."""
    return np.asarray(out, np.float32).reshape(L_, H, E)


def _ensure_fast_setup(nc, n_cores):
    """Build + cache the sharded executable, on-device zeros maker, and
    name/mesh metadata for the fast PJRT path."""
    import jax
    import jax.numpy as jnp
    from jax.experimental.shard_map import shard_map
    from jax.sharding import Mesh, NamedSharding, PartitionSpec
    from concourse import bass2jax, mybir

    bass2jax.install_neuronx_cc_hook()

    key = id(nc)
    if _CACHE.get("fast_key") != key:
        partition_name = (
            nc.partition_id_tensor.name if nc.partition_id_tensor else None
        )
        in_names, out_names, out_avals, zero_shapes = [], [], [], []
        for alloc in nc.m.functions[0].allocations:
            if not isinstance(alloc, mybir.MemoryLocationSet):
                continue
            name = alloc.memorylocations[0].name
            if alloc.kind == "ExternalInput":
                if name != partition_name:
                    in_names.append(name)
            elif alloc.kind == "ExternalOutput":
                shape = tuple(alloc.tensor_shape)
                dtype = mybir.dt.np(alloc.dtype)
                out_names.append(name)
                out_avals.append(jax.core.ShapedArray(shape, dtype))
                zero_shapes.append((shape, dtype))
        n_params = len(in_names)
        n_outs = len(out_avals)
        in_names.extend(out_names)
        if partition_name is not None:
            in_names.append(partition_name)
        donate = tuple(range(n_params, n_params + n_outs))

        def _body(*args):
            operands = list(args)
            if partition_name is not None:
                operands.append(bass2jax.partition_id_tensor())
            outs = bass2jax._bass_exec_p.bind(
                *operands,
                out_avals=tuple(out_avals),
                in_names=tuple(in_names),
                out_names=tuple(out_names),
                lowering_input_output_aliases=(),
                sim_require_finite=True,
                sim_require_nnan=True,
                nc=nc,
            )
            return tuple(outs)

        devices = jax.devices()[:n_cores]
        mesh = Mesh(np.asarray(devices), ("core",))
        sharded = jax.jit(
            shard_map(
                _body,
                mesh=mesh,
                in_specs=(PartitionSpec("core"),) * (n_params + n_outs),
                out_specs=(PartitionSpec("core"),) * n_outs,
                check_rep=False,
            ),
            donate_argnums=donate,
            keep_unused=True,
        )
        zsh = (NamedSharding(mesh, PartitionSpec("core")),) * n_outs
        mk_zeros = jax.jit(
            lambda: tuple(
                jnp.zeros((n_cores * s[0], *s[1:]), d) for s, d in zero_shapes
            ),
            out_shardings=zsh,
        )
        _CACHE.update(
            fast_key=key, fast_sharded=sharded, fast_mk_zeros=mk_zeros,
            fast_names=(in_names, out_names, out_avals, n_params),
            fast_devices=devices, fast_sharding=zsh[0] if zsh else None,
        )


def _fast_run_bass_via_pjrt(nc, in_maps, n_cores):
    """Drop-in for bass2jax.run_bass_via_pjrt (multi-core, no-debug path):
    donated zero output buffers are allocated ON DEVICE, and per-core input
    shards that are already jax arrays (pre-staged asynchronously by
    kernel()) are assembled without a host-side concat."""
    import jax

    if n_cores == 1 or nc.dbg_addr is not None:
        return _CACHE["orig_run"](nc, in_maps, n_cores)

    _ensure_fast_setup(nc, n_cores)
    in_names, out_names, out_avals, n_params = _CACHE["fast_names"]
    sharded = _CACHE["fast_sharded"]
    sharding = _CACHE["fast_sharding"]

    if isinstance(in_maps[0][in_names[0]], jax.Array):
        global_in = []
        for name in in_names[:n_params]:
            shards = [in_maps[c][name] for c in range(n_cores)]
            s0 = shards[0].shape
            global_in.append(
                jax.make_array_from_single_device_arrays(
                    (n_cores * s0[0], *s0[1:]), sharding, shards
                )
            )
    else:
        global_in = [
            np.concatenate(
                [np.asarray(in_maps[c][name]) for c in range(n_cores)], axis=0
            )
            for name in in_names[:n_params]
        ]
    concat_zeros = _CACHE["fast_mk_zeros"]()
    out_arrs = sharded(*global_in, *concat_zeros)
    try:
        # fetch the 8 per-device shards concurrently (~1.3x tunnel speedup)
        from concurrent.futures import ThreadPoolExecutor

        results = [dict() for _ in range(n_cores)]
        for i, name in enumerate(out_names):
            shards = sorted(
                out_arrs[i].addressable_shards,
                key=lambda s: s.index[0].start or 0,
            )
            assert len(shards) == n_cores
            with ThreadPoolExecutor(n_cores) as ex:
                datas = list(ex.map(lambda s: np.asarray(s.data), shards))
            for c in range(n_cores):
                results[c][name] = datas[c]
        return results
    except Exception:
        return [
            {
                name: np.asarray(out_arrs[i]).reshape(
                    n_cores, *out_avals[i].shape
                )[c]
                for i, name in enumerate(out_names)
            }
            for c in range(n_cores)
        ]


def _install_fast_pjrt():
    from concourse import bass2jax

    if "orig_run" not in _CACHE:
        _CACHE["orig_run"] = bass2jax.run_bass_via_pjrt
        bass2jax.run_bass_via_pjrt = _fast_run_bass_via_pjrt


def kernel(queries, keys, values):
    from concourse import bass_utils

    if "nc" not in _CACHE:
        _CACHE["nc"] = build_bass(T)
    nc = _CACHE["nc"]

    in_maps = None
    try:
        # pre-stage: pack each core then start its h2d transfer on a thread
        # pool, so host packing and the (slow, ~1.3x-parallelizable) tunnel
        # transfers overlap
        import jax

        _install_fast_pjrt()
        _ensure_fast_setup(nc, N_CORES)
        devs = _CACHE["fast_devices"]
        in_maps = []
        for b in range(N_CORES):
            m = pack_inputs(
                np.asarray(queries[b]), np.asarray(keys[b]),
                np.asarray(values[b]),
            )
            in_maps.append(
                {k: jax.device_put(v, devs[b]) for k, v in m.items()}
            )
    except Exception:
        in_maps = None

    if in_maps is None:
        in_maps = [
            pack_inputs(
                np.asarray(queries[b]), np.asarray(keys[b]),
                np.asarray(values[b]),
            )
            for b in range(N_CORES)
        ]

    try:
        _install_fast_pjrt()
        res = bass_utils.run_bass_kernel_spmd(
            nc, in_maps, core_ids=list(range(N_CORES))
        )
    except Exception:
        from concourse import bass2jax

        if "orig_run" in _CACHE:  # unpatch and retry on the stock path
            bass2jax.run_bass_via_pjrt = _CACHE["orig_run"]
        res = bass_utils.run_bass_kernel_spmd(
            nc, in_maps, core_ids=list(range(N_CORES))
        )
    out = np.stack(
        [unpack_output(res.results[b]["out"], L) for b in range(N_CORES)]
    )
    _CACHE["last_result"] = res
    return out.reshape(B, L, H, E)


# revision 39
# speedup vs baseline: 1.0531x; 1.0104x over previous
"""Local (sliding-window, causal) attention on 8 Trainium2 NeuronCores.

Problem: B=8, L=4096, H=8, E=64, window NEIGH=128, SPLITS=32 query blocks of
L1=128.  Query q attends keys [q-127, q].

Sharding: batch b -> core b (8 cores, no communication).

Per-core algorithm (streaming over the 32 sequence tiles):
  - Host pre-packs (numpy): Q,K cast to bf16 and transposed to [e, l] layout
    (partition = e for head h at free offset h*128, free = l), V cast to bf16
    with a ones-column appended per head (denominator trick).
  - ST scores computed transposed [m, l] so that softmax needs NO partition
    reduction and P is consumed by the AV matmul WITHOUT a transpose:
       ST = K_tile @ Q_block^T  (PE, bf16, contraction e=64)
       P = exp(ST * 0.125) (ACT, -> bf16)  [no max-subtraction needed: |S|<~8]
       P *= band mask (0/1 bf16; heads 0-3 on DVE, 4-7 on POOL)
       out_aug[l, 0:65] = sum over the two m-tiles of P^T @ [V | ones]  (PE)
       out[l, 0:64] = out_aug[:, 0:64] / out_aug[:, 64]  (DVE, direct from
       PSUM, emitted as fp16 to halve the output DMA)
  - Block j needs key tiles j-1, j; per step t we compute the single matmul
    pair (stationary KT_{t-1}) that yields tile-b scores of block t-1 and
    tile-a scores of block t; block t-1 then completes and is stored.
"""

import numpy as np
import ml_dtypes

B, L, H, E = 8, 4096, 8, 64
NEIGH = 128
P = 128                 # partitions / rows per tile
T = L // P              # 32 sequence tiles
N_CORES = 8
SCALE = 1.0 / np.sqrt(E)
BF = ml_dtypes.bfloat16

_CACHE = {}


def build_bass(nsteps=T):
    """Build + compile the single-core Bass program (SPMD across 8 cores)."""
    from contextlib import ExitStack
    import concourse.bass as bass  # noqa: F401
    import concourse.mybir as mybir
    import concourse.tile as tile
    from concourse import bacc

    f32, bf16 = mybir.dt.float32, mybir.dt.bfloat16
    f16 = mybir.dt.float16
    Exp = mybir.ActivationFunctionType.Exp

    nc = bacc.Bacc(
        "TRN2", target_bir_lowering=False, debug=False, enable_asserts=False
    )
    qkt_d = nc.dram_tensor(
        "qkt", [nsteps, E, 2 * H * P], bf16, kind="ExternalInput"
    ).ap()
    va_d = nc.dram_tensor(
        "va", [nsteps, P, H * (E + 1)], bf16, kind="ExternalInput"
    ).ap()
    mk_d = nc.dram_tensor("mk", [P, 2 * P], bf16, kind="ExternalInput").ap()
    out_d = nc.dram_tensor("out", [nsteps, P, H * E], f16, kind="ExternalOutput").ap()

    with tile.TileContext(nc) as tc:
        with ExitStack() as ctx:
            nc = tc.nc

            const = ctx.enter_context(tc.tile_pool(name="const", bufs=1))
            # multiplicative band mask, replicated per head: [128, H*256] bf16
            # per head: [0:128] tile-b (valid l>=m), [128:256] tile-a (valid l<m)
            mask = const.tile([P, H * 2 * P], bf16, tag="mask")
            mv = mask[:].rearrange("p (r w) -> p r w", r=H)
            # one DMA, source re-read H times via stride-0 broadcast; issued
            # from ACT (busy with its table load anyway) to keep SP and POOL
            # free for the first qkt/va loads
            nc.scalar.dma_start(
                mv[:],
                mk_d[:]
                .rearrange("p (h w) -> p h w", h=1)
                .broadcast_to([P, H, 2 * P]),
            )

            qk = ctx.enter_context(tc.tile_pool(name="qk", bufs=4))
            vp = ctx.enter_context(tc.tile_pool(name="vp", bufs=4))
            pp = ctx.enter_context(tc.tile_pool(name="pp", bufs=3))
            op = ctx.enter_context(tc.tile_pool(name="op", bufs=4))
            rp = ctx.enter_context(tc.tile_pool(name="rp", bufs=4))
            st_ps = ctx.enter_context(tc.tile_pool(name="st", bufs=2, space="PSUM"))
            av_ps = ctx.enter_context(tc.tile_pool(name="av", bufs=2, space="PSUM"))

            qkt_prev = None
            p_prev = None
            va_hist = [None, None]  # [V tile t-1, V tile t-2]

            for t in range(nsteps + 1):
                qkt = va = None
                if t < nsteps:
                    if t == 0:
                        # prefetch BOTH of the first two qkt tiles up front:
                        # qkt0 on SP, qkt1 on POOL ahead of va0, so the two
                        # loads gating the first exp overlap maximally
                        qkt = qk.tile([E, 2 * H * P], bf16, tag="qkt")
                        nc.sync.dma_start(qkt[:], qkt_d[0])
                        qkt1_pre = qk.tile([E, 2 * H * P], bf16, tag="qkt")
                        nc.gpsimd.dma_start(qkt1_pre[:], qkt_d[1])
                    elif t == 1:
                        qkt = qkt1_pre
                    else:
                        qkt = qk.tile([E, 2 * H * P], bf16, tag="qkt")
                        nc.sync.dma_start(qkt[:], qkt_d[t])
                    va = vp.tile([P, H * (E + 1)], bf16, tag="va")
                    nc.gpsimd.dma_start(va[:], va_d[t])

                if t >= 1:
                    # scores for (block t-1 | tile-b) and (block t | tile-a)
                    pt = pp.tile([P, H * 2 * P], bf16, tag="pt")
                    for g in range(2):  # two groups of 4 heads
                        st = st_ps.tile([P, 4 * 2 * P], f32, tag="st")
                        for i in range(4):
                            h = g * 4 + i
                            c0, c1 = h * P, (h + 1) * P
                            lh = qkt_prev[:, H * P + c0 : H * P + c1]
                            nc.tensor.matmul(
                                st[:, i * 2 * P : i * 2 * P + P],
                                lh, qkt_prev[:, c0:c1],
                                start=True, stop=True,
                            )
                            if t < nsteps:
                                nc.tensor.matmul(
                                    st[:, i * 2 * P + P : (i + 1) * 2 * P],
                                    lh, qkt[:, c0:c1],
                                    start=True, stop=True,
                                )
                        if t < nsteps:
                            nc.scalar.activation(
                                pt[:, g * 4 * 2 * P : (g + 1) * 4 * 2 * P],
                                st[:], Exp, scale=float(SCALE),
                            )
                        else:
                            # last step: only tile-b (left) halves were
                            # written; one strided activation per group
                            sv = st[:].rearrange("p (r w) -> p r w", r=4)
                            gv = (
                                pt[:, g * 4 * 2 * P : (g + 1) * 4 * 2 * P]
                                .rearrange("p (r w) -> p r w", r=4)
                            )
                            nc.scalar.activation(
                                gv[:, :, 0:P], sv[:, :, 0:P],
                                Exp, scale=float(SCALE),
                            )
                    # band mask: heads 0-5 on DVE, heads 6-7 on POOL
                    cut = 6 * 2 * P
                    if t < nsteps:
                        nc.vector.tensor_mul(
                            pt[:, 0:cut], pt[:, 0:cut], mask[:, 0:cut]
                        )
                        nc.gpsimd.tensor_mul(
                            pt[:, cut:], pt[:, cut:], mask[:, cut:]
                        )
                    else:
                        pv = pt[:].rearrange("p (r w) -> p r w", r=H)
                        nc.vector.tensor_mul(
                            pv[:, 0:6, 0:P], pv[:, 0:6, 0:P], mv[:, 0:6, 0:P]
                        )
                        nc.gpsimd.tensor_mul(
                            pv[:, 6:H, 0:P], pv[:, 6:H, 0:P], mv[:, 6:H, 0:P]
                        )

                    # AV for block j = t-1  (out_aug per head: 64 V cols + denom)
                    av = av_ps.tile([P, H * P], f32, tag="av")  # head h at h*128
                    for h in range(H):
                        dst = av[:, h * P : h * P + (E + 1)]
                        vs1 = va_hist[0][:, h * (E + 1) : (h + 1) * (E + 1)]
                        if t >= 2:
                            vs2 = va_hist[1][:, h * (E + 1) : (h + 1) * (E + 1)]
                            nc.tensor.matmul(
                                dst, p_prev[:, h * 2 * P + P : (h + 1) * 2 * P],
                                vs2, start=True, stop=False,
                            )
                            nc.tensor.matmul(
                                dst, pt[:, h * 2 * P : h * 2 * P + P],
                                vs1, start=False, stop=True,
                            )
                        else:
                            nc.tensor.matmul(
                                dst, pt[:, h * 2 * P : h * 2 * P + P],
                                vs1, start=True, stop=True,
                            )

                    # out = av[:, 0:64] / av[:, 64], straight from PSUM -> fp16
                    avv = av[:].rearrange("p (h w) -> p h w", h=H)
                    rr = rp.tile([P, H], f32, tag="rr")
                    rrv = rr[:].rearrange("p (h w) -> p h w", w=1)
                    nc.vector.reciprocal(rrv, avv[:, :, E : E + 1])
                    ob = op.tile([P, H * E], f16, tag="ob")
                    obv = ob[:].rearrange("p (h w) -> p h w", h=H)
                    nc.vector.tensor_mul(
                        obv, avv[:, :, 0:E], rrv.broadcast_to([P, H, E])
                    )
                    nc.gpsimd.dma_start(out_d[t - 1], ob[:])
                    p_prev = pt

                if t < nsteps:
                    va_hist = [va, va_hist[0]]
                    qkt_prev = qkt

    nc.compile()
    return nc


def make_mask():
    """[P, 2P] bf16: [0:128] tile-b valid l>=m; [128:256] tile-a valid l<m."""
    m = np.arange(P)[:, None]
    l = np.arange(P)[None, :]
    mb = (l >= m).astype(np.float32)
    ma = (l < m).astype(np.float32)
    return np.concatenate([mb, ma], axis=1).astype(BF)


def pack_inputs(q, k, v):
    """Per-core host repack: q,k,v [L, H, E] f32 -> dict of device inputs."""
    nst = q.shape[0] // P

    def t_pack(x):
        xb = np.ascontiguousarray(
            x.reshape(nst, P, H, E).transpose(0, 3, 2, 1)
        )  # [t, e, h, l]
        return xb.reshape(nst, E, H * P).astype(BF)

    qkt = np.concatenate([t_pack(q), t_pack(k)], axis=-1)
    vb = v.reshape(nst, P, H, E).astype(BF)
    va = np.concatenate(
        [vb, np.ones((nst, P, H, 1), BF)], axis=-1
    ).reshape(nst, P, H * (E + 1))
    return {"qkt": qkt, "va": va, "mk": make_mask()}


def unpack_output(out, L_):
    """Device out [nsteps, P, H*E] fp16 -> [L, H, E] f32."""
    return np.asarray(out, np.float32).reshape(L_, H, E)


def _ensure_fast_setup(nc, n_cores):
    """Build + cache the sharded executable, on-device zeros maker, and
    name/mesh metadata for the fast PJRT path."""
    import jax
    import jax.numpy as jnp
    from jax.experimental.shard_map import shard_map
    from jax.sharding import Mesh, NamedSharding, PartitionSpec
    from concourse import bass2jax, mybir

    bass2jax.install_neuronx_cc_hook()

    key = id(nc)
    if _CACHE.get("fast_key") != key:
        partition_name = (
            nc.partition_id_tensor.name if nc.partition_id_tensor else None
        )
        in_names, out_names, out_avals, zero_shapes = [], [], [], []
        for alloc in nc.m.functions[0].allocations:
            if not isinstance(alloc, mybir.MemoryLocationSet):
                continue
            name = alloc.memorylocations[0].name
            if alloc.kind == "ExternalInput":
                if name != partition_name:
                    in_names.append(name)
            elif alloc.kind == "ExternalOutput":
                shape = tuple(alloc.tensor_shape)
                dtype = mybir.dt.np(alloc.dtype)
                out_names.append(name)
                out_avals.append(jax.core.ShapedArray(shape, dtype))
                zero_shapes.append((shape, dtype))
        n_params = len(in_names)
        n_outs = len(out_avals)
        in_names.extend(out_names)
        if partition_name is not None:
            in_names.append(partition_name)
        donate = tuple(range(n_params, n_params + n_outs))

        def _body(*args):
            operands = list(args)
            if partition_name is not None:
                operands.append(bass2jax.partition_id_tensor())
            outs = bass2jax._bass_exec_p.bind(
                *operands,
                out_avals=tuple(out_avals),
                in_names=tuple(in_names),
                out_names=tuple(out_names),
                lowering_input_output_aliases=(),
                sim_require_finite=True,
                sim_require_nnan=True,
                nc=nc,
            )
            return tuple(outs)

        devices = jax.devices()[:n_cores]
        mesh = Mesh(np.asarray(devices), ("core",))
        sharded = jax.jit(
            shard_map(
                _body,
                mesh=mesh,
                in_specs=(PartitionSpec("core"),) * (n_params + n_outs),
                out_specs=(PartitionSpec("core"),) * n_outs,
                check_rep=False,
            ),
            donate_argnums=donate,
            keep_unused=True,
        )
        zsh = (NamedSharding(mesh, PartitionSpec("core")),) * n_outs
        mk_zeros = jax.jit(
            lambda: tuple(
                jnp.zeros((n_cores * s[0], *s[1:]), d) for s, d in zero_shapes
            ),
            out_shardings=zsh,
        )
        _CACHE.update(
            fast_key=key, fast_sharded=sharded, fast_mk_zeros=mk_zeros,
            fast_names=(in_names, out_names, out_avals, n_params),
            fast_devices=devices, fast_sharding=zsh[0] if zsh else None,
        )


def _fast_run_bass_via_pjrt(nc, in_maps, n_cores):
    """Drop-in for bass2jax.run_bass_via_pjrt (multi-core, no-debug path):
    donated zero output buffers are allocated ON DEVICE, and per-core input
    shards that are already jax arrays (pre-staged asynchronously by
    kernel()) are assembled without a host-side concat."""
    import jax

    if n_cores == 1 or nc.dbg_addr is not None:
        return _CACHE["orig_run"](nc, in_maps, n_cores)

    _ensure_fast_setup(nc, n_cores)
    in_names, out_names, out_avals, n_params = _CACHE["fast_names"]
    sharded = _CACHE["fast_sharded"]
    sharding = _CACHE["fast_sharding"]

    if isinstance(in_maps[0][in_names[0]], jax.Array):
        global_in = []
        for name in in_names[:n_params]:
            shards = [in_maps[c][name] for c in range(n_cores)]
            s0 = shards[0].shape
            global_in.append(
                jax.make_array_from_single_device_arrays(
                    (n_cores * s0[0], *s0[1:]), sharding, shards
                )
            )
    else:
        global_in = [
            np.concatenate(
                [np.asarray(in_maps[c][name]) for c in range(n_cores)], axis=0
            )
            for name in in_names[:n_params]
        ]
    concat_zeros = _CACHE["fast_mk_zeros"]()
    out_arrs = sharded(*global_in, *concat_zeros)
    try:
        # fetch the 8 per-device shards concurrently (~1.3x tunnel speedup)
        from concurrent.futures import ThreadPoolExecutor

        results = [dict() for _ in range(n_cores)]
        for i, name in enumerate(out_names):
            shards = sorted(
                out_arrs[i].addressable_shards,
                key=lambda s: s.index[0].start or 0,
            )
            assert len(shards) == n_cores
            with ThreadPoolExecutor(n_cores) as ex:
                datas = list(ex.map(lambda s: np.asarray(s.data), shards))
            for c in range(n_cores):
                results[c][name] = datas[c]
        return results
    except Exception:
        return [
            {
                name: np.asarray(out_arrs[i]).reshape(
                    n_cores, *out_avals[i].shape
                )[c]
                for i, name in enumerate(out_names)
            }
            for c in range(n_cores)
        ]


def _install_fast_pjrt():
    from concourse import bass2jax

    if "orig_run" not in _CACHE:
        _CACHE["orig_run"] = bass2jax.run_bass_via_pjrt
        bass2jax.run_bass_via_pjrt = _fast_run_bass_via_pjrt


def kernel(queries, keys, values):
    from concourse import bass_utils

    if "nc" not in _CACHE:
        _CACHE["nc"] = build_bass(T)
    nc = _CACHE["nc"]

    in_maps = None
    try:
        # pre-stage: pack each core then start its h2d transfer on a thread
        # pool, so host packing and the (slow, ~1.3x-parallelizable) tunnel
        # transfers overlap
        import jax

        _install_fast_pjrt()
        _ensure_fast_setup(nc, N_CORES)
        devs = _CACHE["fast_devices"]
        in_maps = []
        for b in range(N_CORES):
            m = pack_inputs(
                np.asarray(queries[b]), np.asarray(keys[b]),
                np.asarray(values[b]),
            )
            in_maps.append(
                {k: jax.device_put(v, devs[b]) for k, v in m.items()}
            )
    except Exception:
        in_maps = None

    if in_maps is None:
        in_maps = [
            pack_inputs(
                np.asarray(queries[b]), np.asarray(keys[b]),
                np.asarray(values[b]),
            )
            for b in range(N_CORES)
        ]

    try:
        _install_fast_pjrt()
        res = bass_utils.run_bass_kernel_spmd(
            nc, in_maps, core_ids=list(range(N_CORES))
        )
    except Exception:
        from concourse import bass2jax

        if "orig_run" in _CACHE:  # unpatch and retry on the stock path
            bass2jax.run_bass_via_pjrt = _CACHE["orig_run"]
        res = bass_utils.run_bass_kernel_spmd(
            nc, in_maps, core_ids=list(range(N_CORES))
        )
    out = np.stack(
        [unpack_output(res.results[b]["out"], L) for b in range(N_CORES)]
    )
    _CACHE["last_result"] = res
    return out.reshape(B, L, H, E)
